# revision 1
# baseline (speedup 1.0000x reference)
"""AttentionPool2d Trainium2 kernel (8-core data parallel over batch).

Math (per batch item), exploiting that only query token 0 survives into the
output: tokens t = [mean(x); x_tokens] + pos_emb; v = t @ Wv.T + bv;
out[1:] = v[1:] @ Wc.T + bc; out[0] = softmax(q0.K/sqrt(hd)) V @ Wc.T + bc
with q0 = K = V = v (per head).

Split into two precision domains:
 - tokens 1..49 (98% of the output mass) bypass v entirely:
   out[n] = x_n @ W2.T + pconst[n], W2 = Wc @ Wv (host-precomputed, fp16
   matmul on device). pconst[n] = pos_n @ W2.T + bv @ Wc.T is folded into
   the same PSUM accumulation via a one-hot 17th matmul.
 - token 0 goes through attention, where ~4% relative error is invisible
   in the full-output l2 (weight ~1/50): v is computed with fp8-e4m3
   DoubleRow matmuls (2x PE throughput; scales 32*t and 64*Wv keep
   everything in e4m3 normal range, TRN max 240), attention runs on
   vT' = 2048*v fp16, and out0 = u @ Wc with u,Wc in fp8.

Measured end-to-end l2 vs reference ~8e-4 (budget 2e-2).
"""

import numpy as np

import bass_rust
import concourse.bass as bass
import concourse.mybir as mybir
import concourse.tile as tile
from concourse.bass_utils import run_bass_kernel_spmd
from concourse.tile_scheduler import PROC_NAME_TO_IDX
from contextlib import ExitStack

# ---------------------------------------------------------------- constants
B, C, S = 256, 2048, 7
HW = S * S              # 49 spatial tokens
N = HW + 1              # 50 tokens incl. mean token
H, OUT = 32, 1024       # default num_heads; build is parameterized
HD = C // H
CORES = 8
IPC = B // CORES        # 32 items per core
GI = 8                  # items per group
G = IPC // GI           # 4 groups
NG = GI * N             # 400 moving columns per group
KC = C // 128           # 16 contraction chunks
KC2 = KC // 2           # 8 fp8 DoubleRow super-chunks
JC = C // 128           # 16 output-channel chunks of v
XTOK = IPC * HW         # 1568 spatial tokens per core (x-path)
NT = (XTOK + 127) // 128  # 13 x-path token tiles
OC2 = OUT // 512        # 2 out-projection column chunks

# fp8 scaling: tT8 = 32*t, wv8 = 64*Wv  =>  psum = 2048*(t@Wv.T)
SV = 2048.0             # vT' = SV * v
SU = 32.0               # uT = SU * ctx
SW = 64.0               # wc8 = SW * Wc

F8 = mybir.dt.float8e4
F16 = mybir.dt.float16
F32 = mybir.dt.float32

N_PROCS = 27


# ------------------------------------------------------- tile/walrus patches
def _patched_drain_and_barrier(self, tick_clock, wait_clock):
    """Stock tail drain carries one wait per ticked proc; walrus here allows
    a single sync-wait per instruction. Funnel waits through SP nops."""
    nc = self.nc
    gc = tick_clock.global_clock
    ticks = [gc.peek_next(i) - 1 for i in range(N_PROCS)]
    live = [i for i in range(N_PROCS) if ticks[i] > 0]
    sp_clock = wait_clock.engine_clocks[PROC_NAME_TO_IDX["SP"]]
    for p in live:
        vc = bass_rust.VectorClock()
        vc.require_at_least(p, ticks[p])
        nop = nc.sync.nop(nofuse=True, hint="tail_wait_funnel")
        wait_clock.add_sem_waits(
            nop.ins, bass_rust.ScopedClock({None: vc}), cur_clock=sp_clock
        )
        sp_clock.require_at_least(None, p, ticks[p])
    drain_inst = nc.sync.drain()
    wait_clock.add_sem_waits(
        drain_inst.ins, bass_rust.ScopedClock({None: gc}), cur_clock=sp_clock
    )
    nc.all_engine_barrier()
    assert self.sems is not None
    popped = nc._tile_sem_poison_stack.pop()
    assert popped is self._sem_poison
    nc.clear_and_free_semaphores(list(self.sems.allocated().values()))
    nc.all_engine_barrier()


tile.TileContext._drain_and_barrier = _patched_drain_and_barrier


def fix_excess_waits(nc, max_waits=1):
    """Hoist excess per-instruction sync-waits onto injected same-engine
    NoOps placed immediately before the offender (engine streams run in
    basic-block order)."""
    for bb in nc.m.functions[0].blocks:
        insts = bb.instructions
        if not any(
            i.sync_info and i.sync_info.on_wait and len(i.sync_info.on_wait) > max_waits
            for i in insts
        ):
            continue
        out = []
        for inst in insts:
            si = inst.sync_info
            if si and si.on_wait and len(si.on_wait) > max_waits:
                waits = list(si.on_wait)
                extra, keep = waits[:-max_waits], waits[-max_waits:]
                for i in range(0, len(extra), max_waits):
                    chunk = extra[i : i + max_waits]
                    nop = mybir.InstNoOp(
                        name=nc.get_next_instruction_name(), ins=[], outs=[]
                    )
                    nop.engine = inst.engine
                    nop.sync_info = bass_rust.SyncInfo(on_wait=chunk, on_update=[])
                    nc.register_instruction(nop)
                    out.append(nop)
                si.on_wait = keep
            out.append(inst)
        bb.instructions = out


def dedup_ldweights(nc):
    """Drop an InstLdweights whose weights AP (and modes) match the previous
    weight load on the PE stream — the PE array keeps the stationary operand
    across matmuls, so a reload of identical weights only burns LDW cycles.
    Only loads carrying no sem waits/updates are removed."""
    import concourse.mybir as mb

    for bb in nc.m.functions[0].blocks:
        last = None
        out = []
        for inst in bb.instructions:
            if isinstance(inst, mb.InstLdweights):
                s = (
                    str(inst.ins[0]),
                    str(getattr(inst, "perf_mode", None)),
                    str(getattr(inst, "is_transpose", None)),
                    str(getattr(inst, "tile_position", None)),
                )
                clean = not inst.sync_info or (
                    not inst.sync_info.on_wait and not inst.sync_info.on_update
                )
                if s == last and clean:
                    continue
                last = s
            out.append(inst)
        bb.instructions = out


# ------------------------------------------------------------- kernel build
def build_kernel(reps=1, variant="full", heads=H, unroll=False):
    nc = bass.Bass("TRN2", target_bir_lowering=False, debug=False)

    x_d = nc.dram_tensor("x", [IPC, C, HW], F16, kind="ExternalInput")
    wv8_d = nc.dram_tensor("wv8", [C, C], F8, kind="ExternalInput")
    w2_d = nc.dram_tensor("w2T", [C, OUT], F16, kind="ExternalInput")
    wc8_d = nc.dram_tensor("wc8", [C, OUT], F8, kind="ExternalInput")
    vpos_d = nc.dram_tensor("vposT", [128, KC * N], F32, kind="ExternalInput")
    maskT_d = nc.dram_tensor("maskT", [128, KC * heads], F8, kind="ExternalInput")
    mask2_d = nc.dram_tensor("mask2", [heads, KC * 128], F16, kind="ExternalInput")
    oneh_d = nc.dram_tensor("oneh", [128, NT * 128], F16, kind="ExternalInput")
    pcm_d = nc.dram_tensor("pcm", [128, OUT], F16, kind="ExternalInput")
    # x-path rows land contiguously (token-major, no token-0 gaps); host
    # reassembles [IPC, N, OUT] from outx + out0.
    outx_d = nc.dram_tensor("outx", [XTOK, OUT], F16, kind="ExternalOutput")
    out0_d = nc.dram_tensor("out0", [IPC, OUT], F16, kind="ExternalOutput")

    with tile.TileContext(nc) as tc, ExitStack() as ctx:
        wv_pool = ctx.enter_context(tc.tile_pool(name="wv", bufs=1))
        w2_pool = ctx.enter_context(tc.tile_pool(name="w2", bufs=1))
        wc_pool = ctx.enter_context(tc.tile_pool(name="wc", bufs=1))
        cpool = ctx.enter_context(tc.tile_pool(name="consts", bufs=1))
        xpool = ctx.enter_context(tc.tile_pool(name="xT", bufs=1))
        spool = ctx.enter_context(tc.tile_pool(name="small", bufs=2))
        tpool = ctx.enter_context(tc.tile_pool(name="tT8", bufs=2))
        vpool = ctx.enter_context(tc.tile_pool(name="vT", bufs=2))
        apool = ctx.enter_context(tc.tile_pool(name="attn", bufs=2))
        ppool = ctx.enter_context(tc.tile_pool(name="pp8", bufs=1))
        opool = ctx.enter_context(tc.tile_pool(name="outsb", bufs=2))
        upool = ctx.enter_context(tc.tile_pool(name="uT", bufs=1))
        pv = ctx.enter_context(tc.tile_pool(name="pv", bufs=2, space="PSUM"))
        pS = ctx.enter_context(tc.tile_pool(name="pS", bufs=1, space="PSUM"))
        pA = ctx.enter_context(tc.tile_pool(name="pA", bufs=3, space="PSUM"))
        po = ctx.enter_context(tc.tile_pool(name="po", bufs=2, space="PSUM"))

        # ---- resident weights/constants (loaded outside the rep loop)
        wv8_sb = wv_pool.tile([128, KC * C], F8, name="wv8")
        for kc in range(KC):
            nc.sync.dma_start(
                wv8_sb[:, kc * C : (kc + 1) * C],
                wv8_d.ap()[kc * 128 : (kc + 1) * 128, :],
            )
        w2_sb, wc8_sb = [], []
        for kc in range(KC):
            w = w2_pool.tile([128, OUT], F16, name=f"w2{kc}", tag=f"w2{kc}")
            nc.sync.dma_start(w[:], w2_d.ap()[kc * 128 : (kc + 1) * 128, :])
            w2_sb.append(w)
            w8 = wc_pool.tile([128, OUT], F8, name=f"wc{kc}", tag=f"wc{kc}")
            nc.sync.dma_start(w8[:], wc8_d.ap()[kc * 128 : (kc + 1) * 128, :])
            wc8_sb.append(w8)
        vpos_sb = cpool.tile([128, KC * N], F32, name="vpos")
        nc.sync.dma_start(vpos_sb[:], vpos_d.ap())
        maskT_sb = cpool.tile([128, KC * heads], F8, name="maskT")
        nc.sync.dma_start(maskT_sb[:], maskT_d.ap())
        mask2_sb = cpool.tile([heads, KC * 128], F16, name="mask2")
        nc.sync.dma_start(mask2_sb[:], mask2_d.ap())
        oneh_sb = cpool.tile([128, NT * 128], F16, name="oneh")
        nc.sync.dma_start(oneh_sb[:], oneh_d.ap())
        pcm_sb = cpool.tile([128, OUT], F16, name="pcm")
        nc.sync.dma_start(pcm_sb[:], pcm_d.ap())

        # x tokens resident in [channel, kc-major global token] layout:
        # xT[p, kc, j] = x[item j//49, kc*128+p, j%49], fp16
        xT_sb = xpool.tile([128, KC * XTOK], F16, name="xTall")
        # uT[p, kc, i] = SU * ctx[item i, kc*128+p], fp8
        uT_sb = upool.tile([128, KC * IPC], F8, name="uT")

        def work():
            body(nc, tc, x_d, (outx_d, out0_d), wv8_sb, w2_sb, wc8_sb, vpos_sb,
                 maskT_sb, mask2_sb, oneh_sb, pcm_sb, xT_sb, uT_sb,
                 spool, tpool, vpool, apool, ppool, opool, pv, pS, pA, po,
                 variant, heads)

        if reps == 1:
            work()
        elif unroll:
            for _ in range(reps):
                work()
        else:
            with tc.For_i(0, reps, 1):
                work()

    dedup_ldweights(nc)
    fix_excess_waits(nc)
    return nc


def body(nc, tc, x_d, outs, wv8_sb, w2_sb, wc8_sb, vpos_sb, maskT_sb,
         mask2_sb, oneh_sb, pcm_sb, xT_sb, uT_sb, spool, tpool, vpool,
         apool, ppool, opool, pv, pS, pA, po, variant="full", heads=H):
    outx_d, out0_d = outs
    scale_exp = float((C // heads) ** -0.5)
    wv8_v = wv8_sb[:].rearrange("p (k c) -> p k c", k=KC)
    xT_v = xT_sb[:].rearrange("p (k j) -> p k j", k=KC)
    uT_v = uT_sb[:].rearrange("p (k i) -> p k i", k=KC)
    vpos3 = vpos_sb[:].rearrange("p (k n) -> p k n", k=KC)

    def build_tT8(g):
        # tT8 layout: [128, KC*(GI*N)] fp8 = 32*t, kc-major so the DoubleRow
        # moving operand spans 2 adjacent kc subtiles: [p, 2, 400]
        tT8 = tpool.tile([128, KC * GI * N], F8, name="tT8", tag="tT8")
        t4 = tT8[:].rearrange("p (k i n) -> p k i n", k=KC, i=GI)
        for it in range(GI):
            gi = g * GI + it
            dst = xT_v[:, :, gi * HW : (gi + 1) * HW]
            nc.sync.dma_start(
                dst, x_d.ap()[gi].rearrange("(k p) n -> p k n", p=128)
            )
            # spatial tokens: fp8(32 * x)
            nc.scalar.activation(
                t4[:, :, it, 1:N], dst,
                mybir.ActivationFunctionType.Copy, scale=32.0,
            )
            # mean token: fp8(32/49 * sum_s x)
            xsum = spool.tile([128, KC], F32, name="xsum", tag="xsum")
            nc.vector.reduce_sum(xsum[:], dst, axis=mybir.AxisListType.X)
            nc.scalar.activation(
                t4[:, :, it, 0], xsum[:],
                mybir.ActivationFunctionType.Copy, scale=32.0 / HW,
            )
        return tT8

    def vproj(g, tT8):
        # vT' = SV * v fp16 via fp8 DoubleRow matmuls (contraction 256/chunk).
        # Per jb, also emit the attention prework on DVE/Act so attnS's
        # matmuls have no cross-engine wait: v0s slice + fp8 pprod = v * v0.
        tT8_3 = tT8[:].rearrange("p (k m) -> p k m", k=KC)
        vTg = vpool.tile([128, JC * NG], F16, name="vTg", tag="vTg")
        v4 = vTg[:].rearrange("p (j i n) -> p j i n", j=JC, i=GI)
        v0s = spool.tile([128, JC * GI], F16, name="v0s", tag="v0s")
        v0s4 = v0s[:].rearrange("p (j i o) -> p j i o", j=JC, o=1)
        pp8 = ppool.tile([128, KC * NG], F8, name="pp8", tag="pp8")
        pp8_4 = pp8[:].rearrange("p (j i n) -> p j i n", j=JC, i=GI)
        for jb in range(JC):
            psum = pv.tile([128, NG], F32, name="pvt", tag="pvt")
            for k2 in range(KC2):
                nc.tensor.matmul(
                    psum[:],
                    wv8_v[:, 2 * k2 : 2 * k2 + 2, jb * 128 : (jb + 1) * 128],
                    tT8_3[:, 2 * k2 : 2 * k2 + 2, :],
                    start=(k2 == 0),
                    stop=(k2 == KC2 - 1),
                    perf_mode=mybir.MatmulPerfMode.DoubleRow,
                )
            nc.vector.tensor_add(
                v4[:, jb],
                psum[:].rearrange("p (i n) -> p i n", i=GI),
                vpos3[:, jb : jb + 1, :].broadcast_to((128, GI, N)),
            )
            # v0s = v0/32 (fp16-normal range)
            nc.scalar.activation(
                v0s4[:, jb, :, 0], v4[:, jb, :, 0],
                mybir.ActivationFunctionType.Copy, scale=2.0 ** -16,
            )
            # pprod = (vT'/64) * (v0/32) = v * v0, fp8
            nc.vector.scalar_tensor_tensor(
                pp8_4[:, jb],
                v4[:, jb],
                2.0 ** -6,
                v0s4[:, jb].broadcast_to((128, GI, N)),
                op0=mybir.AluOpType.mult,
                op1=mybir.AluOpType.mult,
            )
        return vTg, pp8

    def attnS(g, vTg, pp8):
        # S per head: fp8 DoubleRow masked matmuls over pprod
        maskT_v = maskT_sb[:].rearrange("p (k h) -> p k h", k=KC)
        pp8_3 = pp8[:].rearrange("p (k m) -> p k m", k=KC)
        psum_S = pS.tile([heads, NG], F32, name="psS", tag="psS")
        for k2 in range(KC2):
            nc.tensor.matmul(
                psum_S[:],
                maskT_v[:, 2 * k2 : 2 * k2 + 2, :],
                pp8_3[:, 2 * k2 : 2 * k2 + 2, :],
                start=(k2 == 0),
                stop=(k2 == KC2 - 1),
                perf_mode=mybir.MatmulPerfMode.DoubleRow,
            )
        return psum_S

    xp_ps = {}

    def xpath_part(ti, part):
        # half of one token tile's contraction (kcs 0-7 or 8-15), both
        # out-column halves; kc-outer so each stationary x-token chunk is
        # loaded once per pair of matmuls. The accumulation group stays open
        # across interleaved matmuls to other PSUM banks (psA), giving the
        # scheduler ~3.6us PE filler units. part 1 finishes with the pconst
        # one-hot matmul, Act-engine copies, and the whole-tile DMA.
        m0 = ti * 128
        mw = min(128, XTOK - m0)
        if part == 0:
            xp_ps[ti] = [po.tile([128, 512], F32, name=f"pso{oc}", tag="pso")
                         for oc in range(OC2)]
        ps = xp_ps[ti]
        for kc in range(8 * part, 8 * part + 8):
            for oc in range(OC2):
                nc.tensor.matmul(
                    ps[oc][:mw, :],
                    xT_v[:, kc, m0 : m0 + mw],
                    w2_sb[kc][:, oc * 512 : (oc + 1) * 512],
                    start=(kc == 0),
                    stop=False,
                    skip_group_check=True,
                )
        if part == 1:
            for oc in range(OC2):
                nc.tensor.matmul(
                    ps[oc][:mw, :],
                    oneh_sb[:, m0 : m0 + mw],
                    pcm_sb[:, oc * 512 : (oc + 1) * 512],
                    start=False,
                    stop=True,
                    skip_group_check=True,
                )
            osb = opool.tile([128, OUT], F16, name="osb", tag="osb")
            for oc in range(OC2):
                nc.scalar.activation(
                    osb[:mw, oc * 512 : (oc + 1) * 512], ps[oc][:mw, :],
                    mybir.ActivationFunctionType.Copy,
                )
            nc.sync.dma_start(outx_d.ap()[m0 : m0 + mw, :], osb[:mw, :])
            del xp_ps[ti]

    def xpath_tile(ti):
        xpath_part(ti, 0)
        xpath_part(ti, 1)

    def attnAV(g, vTg, psum_S, fillers=()):
        # A = softmax(S); u' = SV * sum_m A[h(c), m] v[c, m] -> uT fp8.
        # fillers: x-path oc-blocks interleaved so the PE streams matmuls
        # while the DVE drains each psA bank.
        v3 = vTg[:].rearrange("p (j m) -> p j m", j=JC)
        e_sb = apool.tile([heads, NG], F32, name="esb", tag="esb")
        nc.scalar.activation(
            e_sb[:], psum_S[:], mybir.ActivationFunctionType.Exp,
            scale=scale_exp,
        )
        d_sb = apool.tile([heads, GI], F32, name="dsb", tag="dsb")
        nc.vector.reduce_sum(
            d_sb[:],
            e_sb[:].rearrange("p (i n) -> p i n", i=GI),
            axis=mybir.AxisListType.X,
        )
        r_sb = apool.tile([heads, GI], F32, name="rsb", tag="rsb")
        nc.vector.reciprocal(r_sb[:], d_sb[:])
        a_sb = apool.tile([heads, NG], F16, name="asb", tag="asb")
        nc.vector.tensor_mul(
            a_sb[:].rearrange("p (i n) -> p i n", i=GI),
            e_sb[:].rearrange("p (i n) -> p i n", i=GI),
            r_sb[:].rearrange("p (i o) -> p i o", o=1).broadcast_to((heads, GI, N)),
        )
        fillers = list(fillers)
        chunks = [range(0, 3), range(3, 6), range(6, 9), range(9, 12),
                  range(12, 15), range(15, 16)]
        for ci, chunk in enumerate(chunks):
            for jb in chunk:
                psum_a = pA.tile([128, NG], F32, name="psA", tag="psA")
                nc.tensor.matmul(
                    psum_a[:],
                    mask2_sb[:, jb * 128 : (jb + 1) * 128],
                    a_sb[:],
                    start=True,
                    stop=True,
                )
                p2 = apool.tile([128, NG], F16, name="p2", tag="p2")
                nc.vector.tensor_mul(p2[:], psum_a[:], v3[:, jb])
                ctx8 = apool.tile([128, GI], F32, name="ctx8", tag="ctx8")
                nc.vector.reduce_sum(
                    ctx8[:],
                    p2[:].rearrange("p (i n) -> p i n", i=GI),
                    axis=mybir.AxisListType.X,
                )
                nc.scalar.activation(
                    uT_v[:, jb, g * GI : (g + 1) * GI], ctx8[:],
                    mybir.ActivationFunctionType.Copy, scale=SU / SV,
                )
            if ci < len(fillers):
                xpath_part(*fillers[ci])
        for fi in range(len(chunks), len(fillers)):
            xpath_part(*fillers[fi])

    def out0proj():
        # out0 = u @ Wc.T: psum = (SU*ctx)@(SW*Wc) -> scale 1/(SU*SW)
        o0 = opool.tile([IPC, OUT], F16, name="o0sb", tag="o0sb")
        for oc in range(OC2):
            psum = po.tile([128, 512], F32, name="ps0", tag="pso")
            for kc in range(KC):
                nc.tensor.matmul(
                    psum[:IPC, :],
                    uT_v[:, kc, :],
                    wc8_sb[kc][:, oc * 512 : (oc + 1) * 512],
                    start=(kc == 0),
                    stop=(kc == KC - 1),
                )
            nc.scalar.activation(
                o0[:, oc * 512 : (oc + 1) * 512], psum[:IPC, :],
                mybir.ActivationFunctionType.Copy, scale=1.0 / (SU * SW),
            )
        nc.sync.dma_start(out0_d.ap()[:], o0[:])

    # ---- schedule: fp8 vproj / attention pipelined with fp16 x-path tiles
    if variant == "full":
        vt0 = vproj(0, build_tT8(0))
        s_prev = attnS(0, *vt0)
        v_prev = vt0[0]
        for g in range(1, G + 1):
            if g < G:
                vtg = vproj(g, build_tT8(g))
            tiles = [3 * (g - 1) + t for t in range(3)]
            if g == G:
                tiles.append(12)
            fillers = [(t, p) for t in tiles for p in range(2)]
            attnAV(g - 1, v_prev, s_prev, fillers)
            if g < G:
                s_prev = attnS(g, *vtg)
                v_prev = vtg[0]
        out0proj()
    elif variant == "vproj":
        for g in range(G):
            vproj(g, build_tT8(g))
    elif variant in ("xpath", "xmm", "xnodma"):
        for it in range(IPC):
            nc.sync.dma_start(
                xT_v[:, :, it * HW : (it + 1) * HW],
                x_d.ap()[it].rearrange("(k p) n -> p k n", p=128),
            )
        if variant == "xmm":
            # sim bisect: matmuls only, no psum drain
            for ti in range(NT):
                for oc in range(OC2):
                    m0 = ti * 128
                    mw = min(128, XTOK - m0)
                    psum = po.tile([128, 512], F32, name="pso", tag="pso")
                    for kc in range(KC):
                        nc.tensor.matmul(
                            psum[:mw, :],
                            xT_v[:, kc, m0 : m0 + mw],
                            w2_sb[kc][:, oc * 512 : (oc + 1) * 512],
                            start=(kc == 0),
                            stop=(kc == KC - 1),
                        )
        elif variant == "xnodma":
            # sim bisect: matmuls + copies, no out DMA
            for ti in range(NT):
                for oc in range(OC2):
                    m0 = ti * 128
                    mw = min(128, XTOK - m0)
                    psum = po.tile([128, 512], F32, name="pso", tag="pso")
                    for kc in range(KC):
                        nc.tensor.matmul(
                            psum[:mw, :],
                            xT_v[:, kc, m0 : m0 + mw],
                            w2_sb[kc][:, oc * 512 : (oc + 1) * 512],
                            start=(kc == 0),
                            stop=(kc == KC - 1),
                        )
                    osb = opool.tile([128, OUT], F16, name="osb", tag="osb")
                    nc.scalar.activation(
                        osb[:mw, oc * 512 : (oc + 1) * 512], psum[:mw, :],
                        mybir.ActivationFunctionType.Copy,
                    )
        else:
            for ti in range(NT):
                xpath_tile(ti)
    elif variant == "attn":
        for g in range(G):
            vTg, v0s = vproj(g, build_tT8(g))
            attnAV(g, vTg, attnS(g, vTg, v0s))
        out0proj()


_NC_CACHE = {}
_RUN_CACHE = {}


def _get_nc(heads):
    if heads not in _NC_CACHE:
        _NC_CACHE[heads] = build_kernel(heads=heads)
    return _NC_CACHE[heads]


def _run(nc, in_maps):
    """run_bass_kernel_spmd equivalent (axon/PJRT path) with: the jitted
    executable cached across calls, weight-like inputs passed replicated
    (uploaded once, not 8x), and donated output buffers created on device
    (no zero upload)."""
    import jax
    import jax.numpy as jnp
    import numpy as _np
    from jax.sharding import Mesh, PartitionSpec, NamedSharding
    from jax.experimental.shard_map import shard_map
    import concourse.mybir as mb
    from concourse import bass2jax as b2j

    # inputs where every core got the identical array object -> replicated
    replicated = {
        nm
        for nm in in_maps[0]
        if all(m[nm] is in_maps[0][nm] for m in in_maps)
    }

    key = id(nc)
    if key not in _RUN_CACHE:
        b2j.install_neuronx_cc_hook()
        in_names, out_names, out_avals = [], [], []
        partition_name = (
            nc.partition_id_tensor.name if nc.partition_id_tensor else None
        )
        for alloc in nc.m.functions[0].allocations:
            if not isinstance(alloc, mb.MemoryLocationSet):
                continue
            name = alloc.memorylocations[0].name
            if alloc.kind == "ExternalInput":
                if name != partition_name:
                    in_names.append(name)
            elif alloc.kind == "ExternalOutput":
                shape = tuple(alloc.tensor_shape)
                dtype = mb.dt.np(alloc.dtype)
                out_names.append(name)
                out_avals.append(jax.core.ShapedArray(shape, dtype))
        n_params = len(in_names)
        n_outs = len(out_avals)
        all_names = list(in_names) + list(out_names)
        if partition_name is not None:
            all_names.append(partition_name)
        donate = tuple(range(n_params, n_params + n_outs))

        def _body(*args):
            operands = list(args)
            if partition_name is not None:
                operands.append(b2j.partition_id_tensor())
            outs = b2j._bass_exec_p.bind(
                *operands,
                out_avals=tuple(out_avals),
                in_names=tuple(all_names),
                out_names=tuple(out_names),
                lowering_input_output_aliases=(),
                sim_require_finite=True,
                sim_require_nnan=True,
                nc=nc,
            )
            return tuple(outs)

        devices = jax.devices()[:CORES]
        mesh = Mesh(_np.asarray(devices), ("core",))
        in_specs = tuple(
            PartitionSpec() if nm in replicated else PartitionSpec("core")
            for nm in in_names
        ) + (PartitionSpec("core"),) * n_outs
        out_specs = (PartitionSpec("core"),) * n_outs
        sharded = jax.jit(
            shard_map(
                _body, mesh=mesh, in_specs=in_specs, out_specs=out_specs,
                check_rep=False,
            ),
            donate_argnums=donate,
            keep_unused=True,
        )
        zeros_fns = [
            jax.jit(
                (lambda shape, dtype: lambda: jnp.zeros(shape, dtype))(
                    (CORES * av.shape[0], *av.shape[1:]), av.dtype
                ),
                out_shardings=NamedSharding(mesh, PartitionSpec("core")),
            )
            for av in out_avals
        ]
        _RUN_CACHE[key] = (
            sharded, in_names, out_names, out_avals, zeros_fns, replicated
        )

    sharded, in_names, out_names, out_avals, zeros_fns, replicated_c = (
        _RUN_CACHE[key]
    )
    assert replicated == replicated_c, "replication pattern changed"
    args = [
        _np.asarray(in_maps[0][nm])
        if nm in replicated
        else _np.concatenate([_np.asarray(m[nm]) for m in in_maps], axis=0)
        for nm in in_names
    ]
    dev_zeros = [f() for f in zeros_fns]
    out_arrs = sharded(*args, *dev_zeros)
    return [
        {
            nm: _np.asarray(out_arrs[i]).reshape(CORES, *out_avals[i].shape)[c]
            for i, nm in enumerate(out_names)
        }
        for c in range(CORES)
    ]


# ---------------------------------------------------------------- host side
def _fp8(a):
    f8np = mybir.dt.np(F8)  # ml_dtypes.float8_e4m3 (TRN range, max 240)
    return np.clip(a, -240.0, 240.0).astype(f8np)


def make_in_maps(inputs, heads=H):
    x = np.asarray(inputs["x"], np.float32)
    pos_emb = np.asarray(inputs["pos_emb"], np.float32)
    Wv = np.asarray(inputs["Wv"], np.float32)
    bv = np.asarray(inputs["bv"], np.float32)
    Wc = np.asarray(inputs["Wc"], np.float32)
    bc = np.asarray(inputs["bc"], np.float32)
    num_heads = int(np.asarray(inputs["num_heads"]))
    assert num_heads == heads and x.shape == (B, C, S, S)
    assert 1 <= heads <= 128 and C % heads == 0

    wv8 = _fp8(64.0 * Wv.T)                       # [C(k), C(c)]
    W2 = Wc @ Wv                                  # [OUT, C]
    w2T = np.ascontiguousarray(W2.T).astype(np.float16)   # [C, OUT]
    wc8 = _fp8(SW * Wc.T)                         # [C, OUT]

    # vposT[128, kc*50 + n] = SV * (pos_emb @ Wv.T + bv).T chunk-tiled
    vpos = SV * (pos_emb @ Wv.T + bv).astype(np.float32)  # [N, C]
    vposT = np.empty((128, KC * N), np.float32)
    for kc in range(KC):
        vposT[:, kc * N : (kc + 1) * N] = vpos[:, kc * 128 : (kc + 1) * 128].T

    # maskT[p, kc*heads + h] = 1 if channel kc*128+p belongs to head h
    head_of = np.arange(C) // (C // heads)
    maskT = np.zeros((128, KC * heads), mybir.dt.np(F8))
    mask2 = np.zeros((heads, KC * 128), np.float16)
    for kc in range(KC):
        for p in range(128):
            h = head_of[kc * 128 + p]
            maskT[p, kc * heads + h] = 1.0
            mask2[h, kc * 128 + p] = 1.0

    # x-path pos constant: out[n>=1] += pconst[n] via one-hot matmul
    # oneh[p, j] = 1 iff p == j % 49; pcm[p] = pconst[p+1]
    oneh = np.zeros((128, NT * 128), np.float16)
    j = np.arange(XTOK)
    oneh[j % HW, j] = 1.0
    pconst = pos_emb @ W2.T + bv @ Wc.T           # [N, OUT]
    pcm = np.zeros((128, OUT), np.float16)
    pcm[:HW] = pconst[1:].astype(np.float16)

    xr16 = np.ascontiguousarray(x.reshape(B, C, HW).astype(np.float16))
    in_maps = []
    for core in range(CORES):
        in_maps.append(
            {
                "x": xr16[core * IPC : (core + 1) * IPC],
                "wv8": wv8,
                "w2T": w2T,
                "wc8": wc8,
                "vposT": vposT,
                "maskT": maskT,
                "mask2": mask2,
                "oneh": oneh,
                "pcm": pcm,
            }
        )

    return in_maps


def kernel(**inputs):
    from concourse._compat import axon_active

    heads = int(np.asarray(inputs["num_heads"]))
    in_maps = make_in_maps(inputs, heads)
    nc = _get_nc(heads)
    if axon_active():
        results = _run(nc, in_maps)
    else:
        results = run_bass_kernel_spmd(nc, in_maps, list(range(CORES))).results
    out = np.empty((B, N, OUT), np.float32)
    for i in range(CORES):
        blk = out[i * IPC : (i + 1) * IPC]
        blk[:, 1:] = np.asarray(results[i]["outx"]).reshape(IPC, HW, OUT)
        blk[:, 0] = np.asarray(results[i]["out0"])
    bc = np.asarray(inputs["bc"], np.float32)
    if bc.any():
        out = out + bc[None, None, :]
    return out



# revision 2
# speedup vs baseline: 1.0809x; 1.0809x over previous
"""AttentionPool2d Trainium2 kernel (8-core data parallel over batch).

Math (per batch item), exploiting that only query token 0 survives into the
output: tokens t = [mean(x); x_tokens] + pos_emb; v = t @ Wv.T + bv;
out[1:] = v[1:] @ Wc.T + bc; out[0] = softmax(q0.K/sqrt(hd)) V @ Wc.T + bc
with q0 = K = V = v (per head).

Two precision domains:
 - tokens 1..49 (98% of output mass) bypass v: out[n] = x_n @ W2.T, with
   W2 = Wc @ Wv host-precomputed, run as fp16 matmuls; the additive
   pconst[n] = pos_n @ W2.T + bv @ Wc.T and bias bc are added on the HOST
   after gather (exact fp32, frees PE/DVE work).
 - token 0 goes through attention entirely in fp8: the DR-interleaved
   fp8 token tensor t8 = fp8(32*t) is HOST-prepared (incl. mean token) and
   DMA'd in one contiguous shot per group; v' = 48*v is stored fp8
   (e4m3); S = q0.K via a masked fp8-DR matmul over pprod = v*v0;
   out0 = u @ Wc with u, Wc fp8 DoubleRow.

Engine split per 8-item group: PE does vproj DR matmuls + attnS + attnAV
broadcast matmuls + x-path fp16 matmuls (as fillers); DVE does psum
drains (v', pprod, p2/ctx reduces, half the x-path drains); ACT does
exp, small copies and the other half of the x-path drains.
"""

import numpy as np

import bass_rust
import concourse.bass as bass
import concourse.mybir as mybir
import concourse.tile as tile
from concourse.bass_utils import run_bass_kernel_spmd
from concourse.tile_scheduler import PROC_NAME_TO_IDX
from contextlib import ExitStack

# ---------------------------------------------------------------- constants
B, C, S = 256, 2048, 7
HW = S * S              # 49 spatial tokens
N = HW + 1              # 50 tokens incl. mean token
H, OUT = 32, 1024       # default num_heads; build is parameterized
HD = C // H
CORES = 8
IPC = B // CORES        # 32 items per core
GI = 8                  # items per group
G = IPC // GI           # 4 groups
NG = GI * N             # 400 moving columns per group
KC = C // 128           # 16 contraction chunks
KC2 = KC // 2           # 8 fp8 DoubleRow super-chunks
JC = C // 128           # 16 output-channel chunks of v
XTOK = IPC * HW         # 1568 spatial tokens per core (x-path)
NT = (XTOK + 127) // 128  # 13 x-path token tiles
NTA = 7                 # token tiles in the first x half-tensor (7*128=896)
XA = NTA * 128          # 896 tokens in half A
XB = XTOK - XA          # 672 tokens in half B
OC2 = OUT // 512        # 2 out-projection column chunks

# scales: t8 = 32*t, wv8 = 64*Wv  =>  psum = 2048*v;  vT8 = 48*v fp8;
# uT = 32*ctx fp8; wc8 = 64*Wc fp8 => out0 psum = 2048*(ctx@Wc.T)
PS = 2048.0
SV = 48.0
SU = 32.0
SW = 64.0

F8 = mybir.dt.float8e4
F16 = mybir.dt.float16
F32 = mybir.dt.float32

N_PROCS = 27


# ------------------------------------------------------- tile/walrus patches
def _patched_drain_and_barrier(self, tick_clock, wait_clock):
    """Stock tail drain carries one wait per ticked proc; walrus here allows
    a single sync-wait per instruction. Funnel waits through SP nops."""
    nc = self.nc
    gc = tick_clock.global_clock
    ticks = [gc.peek_next(i) - 1 for i in range(N_PROCS)]
    live = [i for i in range(N_PROCS) if ticks[i] > 0]
    sp_clock = wait_clock.engine_clocks[PROC_NAME_TO_IDX["SP"]]
    for p in live:
        vc = bass_rust.VectorClock()
        vc.require_at_least(p, ticks[p])
        nop = nc.sync.nop(nofuse=True, hint="tail_wait_funnel")
        wait_clock.add_sem_waits(
            nop.ins, bass_rust.ScopedClock({None: vc}), cur_clock=sp_clock
        )
        sp_clock.require_at_least(None, p, ticks[p])
    drain_inst = nc.sync.drain()
    wait_clock.add_sem_waits(
        drain_inst.ins, bass_rust.ScopedClock({None: gc}), cur_clock=sp_clock
    )
    nc.all_engine_barrier()
    assert self.sems is not None
    popped = nc._tile_sem_poison_stack.pop()
    assert popped is self._sem_poison
    nc.clear_and_free_semaphores(list(self.sems.allocated().values()))
    nc.all_engine_barrier()


tile.TileContext._drain_and_barrier = _patched_drain_and_barrier


def fix_excess_waits(nc, max_waits=1):
    """Hoist excess per-instruction sync-waits onto injected same-engine
    NoOps placed immediately before the offender (engine streams run in
    basic-block order)."""
    for bb in nc.m.functions[0].blocks:
        insts = bb.instructions
        if not any(
            i.sync_info and i.sync_info.on_wait and len(i.sync_info.on_wait) > max_waits
            for i in insts
        ):
            continue
        out = []
        for inst in insts:
            si = inst.sync_info
            if si and si.on_wait and len(si.on_wait) > max_waits:
                waits = list(si.on_wait)
                extra, keep = waits[:-max_waits], waits[-max_waits:]
                for i in range(0, len(extra), max_waits):
                    chunk = extra[i : i + max_waits]
                    nop = mybir.InstNoOp(
                        name=nc.get_next_instruction_name(), ins=[], outs=[]
                    )
                    nop.engine = inst.engine
                    nop.sync_info = bass_rust.SyncInfo(on_wait=chunk, on_update=[])
                    nc.register_instruction(nop)
                    out.append(nop)
                si.on_wait = keep
            out.append(inst)
        bb.instructions = out


def dedup_ldweights(nc):
    """Drop an InstLdweights whose weights AP (and modes) match the previous
    weight load on the PE stream — the PE array keeps the stationary operand
    across matmuls, so a reload of identical weights only burns LDW cycles.
    Only loads carrying no sem waits/updates are removed."""
    import concourse.mybir as mb

    for bb in nc.m.functions[0].blocks:
        last = None
        out = []
        for inst in bb.instructions:
            if isinstance(inst, mb.InstLdweights):
                s = (
                    str(inst.ins[0]),
                    str(getattr(inst, "perf_mode", None)),
                    str(getattr(inst, "is_transpose", None)),
                    str(getattr(inst, "tile_position", None)),
                )
                clean = not inst.sync_info or (
                    not inst.sync_info.on_wait and not inst.sync_info.on_update
                )
                if s == last and clean:
                    continue
                last = s
            out.append(inst)
        bb.instructions = out


# ------------------------------------------------------------- kernel build
def build_kernel(reps=1, variant="full", heads=H, unroll=False):
    nc = bass.Bass("TRN2", target_bir_lowering=False, debug=False)

    # all host-pre-arranged to [128-partition, contiguous-free] layouts
    xa_d = nc.dram_tensor("xa", [128, KC * XA], F16, kind="ExternalInput")
    xb_d = nc.dram_tensor("xb", [128, KC * XB], F16, kind="ExternalInput")
    t8_d = nc.dram_tensor("t8", [128, G * KC * NG], F8, kind="ExternalInput")
    wv8_d = nc.dram_tensor("wv8", [128, KC * C], F8, kind="ExternalInput")
    w2_d = nc.dram_tensor("w2T", [128, KC * OUT], F16, kind="ExternalInput")
    wc8_d = nc.dram_tensor("wc8", [128, KC * OUT], F8, kind="ExternalInput")
    vpos_d = nc.dram_tensor("vposT", [128, KC * N], F16, kind="ExternalInput")
    maskT_d = nc.dram_tensor("maskT", [128, KC * heads], F8, kind="ExternalInput")
    mask2_d = nc.dram_tensor("mask2", [heads, KC * 128], F16, kind="ExternalInput")
    # x-path rows land contiguously (token-major, no token-0 gaps); host
    # reassembles [IPC, N, OUT] from outx + out0 and adds pconst + bc.
    outx_d = nc.dram_tensor("outx", [XTOK, OUT], F16, kind="ExternalOutput")
    out0_d = nc.dram_tensor("out0", [IPC, OUT], F16, kind="ExternalOutput")

    with tile.TileContext(nc) as tc, ExitStack() as ctx:
        wpool = ctx.enter_context(tc.tile_pool(name="weights", bufs=1))
        cpool = ctx.enter_context(tc.tile_pool(name="consts", bufs=1))
        xpool = ctx.enter_context(tc.tile_pool(name="xT", bufs=1))
        spool = ctx.enter_context(tc.tile_pool(name="small", bufs=2))
        tpool = ctx.enter_context(tc.tile_pool(name="t8", bufs=2))
        vpool = ctx.enter_context(tc.tile_pool(name="vT", bufs=3))
        apool = ctx.enter_context(tc.tile_pool(name="attn", bufs=2))
        ppool = ctx.enter_context(tc.tile_pool(name="pp8", bufs=1))
        opool = ctx.enter_context(tc.tile_pool(name="outsb", bufs=2))
        upool = ctx.enter_context(tc.tile_pool(name="uT", bufs=1))
        pv = ctx.enter_context(tc.tile_pool(name="pv", bufs=2, space="PSUM"))
        pas = ctx.enter_context(tc.tile_pool(name="pAS", bufs=4, space="PSUM"))
        po = ctx.enter_context(tc.tile_pool(name="po", bufs=2, space="PSUM"))

        # ---- resident weights/constants (loaded outside the rep loop)
        wv8_sb = wpool.tile([128, KC * C], F8, name="wv8")
        nc.sync.dma_start(wv8_sb[:], wv8_d.ap())
        w2_sb = wpool.tile([128, KC * OUT], F16, name="w2")
        nc.sync.dma_start(w2_sb[:], w2_d.ap())
        wc8_sb = wpool.tile([128, KC * OUT], F8, name="wc8")
        nc.sync.dma_start(wc8_sb[:], wc8_d.ap())
        vpos_sb = cpool.tile([128, KC * N], F16, name="vpos")
        nc.sync.dma_start(vpos_sb[:], vpos_d.ap())
        maskT_sb = cpool.tile([128, KC * heads], F8, name="maskT")
        nc.sync.dma_start(maskT_sb[:], maskT_d.ap())
        mask2_sb = cpool.tile([heads, KC * 128], F16, name="mask2")
        nc.sync.dma_start(mask2_sb[:], mask2_d.ap())

        # x tokens resident in [channel, kc-major global token] layout, as
        # two tensors split at the 896-token tile boundary so the next rep's
        # DMA only waits on that half's last matmul read.
        xTa_sb = xpool.tile([128, KC * XA], F16, name="xTa")
        xTb_sb = xpool.tile([128, KC * XB], F16, name="xTb")
        # uT[p, kc, i] = SU * ctx[item i, kc*128+p], fp8
        uT_sb = upool.tile([128, KC * IPC], F8, name="uT")

        def work():
            body(nc, tc, xa_d, xb_d, t8_d, (outx_d, out0_d), wv8_sb, w2_sb,
                 wc8_sb, vpos_sb, maskT_sb, mask2_sb, xTa_sb, xTb_sb, uT_sb,
                 spool, tpool, vpool, apool, ppool, opool, pv, pas, po,
                 variant, heads)

        if reps == 1:
            work()
        elif unroll:
            for _ in range(reps):
                work()
        else:
            with tc.For_i(0, reps, 1):
                work()

    dedup_ldweights(nc)
    fix_excess_waits(nc)
    return nc


def body(nc, tc, xa_d, xb_d, t8_d, outs, wv8_sb, w2_sb, wc8_sb, vpos_sb,
         maskT_sb, mask2_sb, xTa_sb, xTb_sb, uT_sb, spool, tpool, vpool,
         apool, ppool, opool, pv, pas, po, variant="full", heads=H):
    outx_d, out0_d = outs
    scale_exp = float((C // heads) ** -0.5)
    wv8_v = wv8_sb[:].rearrange("p (k c) -> p k c", k=KC)
    w2_v = w2_sb[:].rearrange("p (k o) -> p k o", k=KC)
    wc8_v = wc8_sb[:].rearrange("p (k o) -> p k o", k=KC)
    xTa_v = xTa_sb[:].rearrange("p (k j) -> p k j", k=KC)
    xTb_v = xTb_sb[:].rearrange("p (k j) -> p k j", k=KC)
    uT_v = uT_sb[:].rearrange("p (k i) -> p k i", k=KC)
    vpos3 = vpos_sb[:].rearrange("p (k n) -> p k n", k=KC)

    def load_x():
        nc.sync.dma_start(xTa_sb[:], xa_d.ap())
        nc.sync.dma_start(xTb_sb[:], xb_d.ap())

    def load_t8(g):
        t8 = tpool.tile([128, KC * NG], F8, name="t8g", tag="t8g")
        nc.sync.dma_start(t8[:], t8_d.ap()[:, g * KC * NG : (g + 1) * KC * NG])
        return t8

    def vproj(g, t8):
        # vT8 = 48*v fp8 via fp8 DoubleRow matmuls (contraction 256/chunk).
        # Also emit on DVE the attention prework so attnS's matmuls have no
        # cross-engine wait: v0s slice + fp8 pprod = v * v0.
        t8_3 = t8[:].rearrange("p (k m) -> p k m", k=KC)
        vTg = vpool.tile([128, JC * NG], F8, name="vTg", tag="vTg")
        v4 = vTg[:].rearrange("p (j i n) -> p j i n", j=JC, i=GI)
        v0s = spool.tile([128, JC * GI], F16, name="v0s", tag="v0s")
        v0s4 = v0s[:].rearrange("p (j i o) -> p j i o", j=JC, o=1)
        pp8 = ppool.tile([128, KC * NG], F8, name="pp8", tag="pp8")
        pp8_4 = pp8[:].rearrange("p (j i n) -> p j i n", j=JC, i=GI)
        for jb in range(JC):
            psum = pv.tile([128, NG], F32, name="pvt", tag="pvt")
            for k2 in range(KC2):
                nc.tensor.matmul(
                    psum[:],
                    wv8_v[:, 2 * k2 : 2 * k2 + 2, jb * 128 : (jb + 1) * 128],
                    t8_3[:, 2 * k2 : 2 * k2 + 2, :],
                    start=(k2 == 0),
                    stop=(k2 == KC2 - 1),
                    perf_mode=mybir.MatmulPerfMode.DoubleRow,
                )
            # vT8 = psum * (SV/PS) + vpos48 (host folded 48x into vpos)
            nc.vector.scalar_tensor_tensor(
                v4[:, jb],
                psum[:].rearrange("p (i n) -> p i n", i=GI),
                SV / PS,
                vpos3[:, jb : jb + 1, :].broadcast_to((128, GI, N)),
                op0=mybir.AluOpType.mult,
                op1=mybir.AluOpType.add,
            )
            # v0s = v0 (true scale, fp16)
            nc.scalar.activation(
                v0s4[:, jb, :, 0], v4[:, jb, :, 0],
                mybir.ActivationFunctionType.Copy, scale=1.0 / SV,
            )
            # pprod = (vT8/48) * v0 = v * v0, fp8
            nc.vector.scalar_tensor_tensor(
                pp8_4[:, jb],
                v4[:, jb],
                1.0 / SV,
                v0s4[:, jb].broadcast_to((128, GI, N)),
                op0=mybir.AluOpType.mult,
                op1=mybir.AluOpType.mult,
            )
        return vTg, pp8

    def attnS(g, vTg, pp8):
        # S per head: fp8 DoubleRow masked matmuls over pprod
        maskT_v = maskT_sb[:].rearrange("p (k h) -> p k h", k=KC)
        pp8_3 = pp8[:].rearrange("p (k m) -> p k m", k=KC)
        psum_S = pas.tile([heads, NG], F32, name="psS", tag="pAS")
        for k2 in range(KC2):
            nc.tensor.matmul(
                psum_S[:],
                maskT_v[:, 2 * k2 : 2 * k2 + 2, :],
                pp8_3[:, 2 * k2 : 2 * k2 + 2, :],
                start=(k2 == 0),
                stop=(k2 == KC2 - 1),
                perf_mode=mybir.MatmulPerfMode.DoubleRow,
            )
        return psum_S

    def xstat(kc, m0, mw):
        if m0 < XA:
            return xTa_v[:, kc, m0 : m0 + mw]
        return xTb_v[:, kc, m0 - XA : m0 - XA + mw]

    xp_ps = {}

    def xpath_part(ti, part):
        # half of one token tile's contraction (kcs 0-7 or 8-15), both
        # out-column halves; kc-outer so each stationary x-token chunk is
        # loaded once per pair of matmuls. The accumulation group stays open
        # across interleaved matmuls to other PSUM banks, giving the
        # scheduler ~3.4us PE filler units. part 1 finishes with split
        # psum drains (oc0 on DVE, oc1 on ACT) and the whole-tile DMA.
        m0 = ti * 128
        mw = min(128, XTOK - m0)
        if part == 0:
            xp_ps[ti] = [po.tile([128, 512], F32, name=f"pso{oc}", tag="pso")
                         for oc in range(OC2)]
        ps = xp_ps[ti]
        for kc in range(8 * part, 8 * part + 8):
            for oc in range(OC2):
                nc.tensor.matmul(
                    ps[oc][:mw, :],
                    xstat(kc, m0, mw),
                    w2_v[:, kc, oc * 512 : (oc + 1) * 512],
                    start=(kc == 0),
                    stop=(part == 1 and kc == 15),
                    skip_group_check=True,
                )
        if part == 1:
            osb = opool.tile([128, OUT], F16, name="osb", tag="osb")
            nc.vector.tensor_copy(osb[:mw, 0:512], ps[0][:mw, :])
            nc.scalar.activation(
                osb[:mw, 512:1024], ps[1][:mw, :],
                mybir.ActivationFunctionType.Copy,
            )
            nc.sync.dma_start(outx_d.ap()[m0 : m0 + mw, :], osb[:mw, :])
            del xp_ps[ti]

    def xpath_tile(ti):
        xpath_part(ti, 0)
        xpath_part(ti, 1)

    def attnAV(g, vTg, psum_S, fillers=()):
        # A = softmax(S); uT = SU * sum_m A[h(c), m] v[c, m] fp8.
        # fillers: x-path oc-blocks interleaved so the PE streams matmuls
        # while the DVE drains each pAS bank.
        v3 = vTg[:].rearrange("p (j m) -> p j m", j=JC)
        e_sb = apool.tile([heads, NG], F32, name="esb", tag="esb")
        nc.scalar.activation(
            e_sb[:], psum_S[:], mybir.ActivationFunctionType.Exp,
            scale=scale_exp,
        )
        d_sb = apool.tile([heads, GI], F32, name="dsb", tag="dsb")
        nc.vector.reduce_sum(
            d_sb[:],
            e_sb[:].rearrange("p (i n) -> p i n", i=GI),
            axis=mybir.AxisListType.X,
        )
        r_sb = apool.tile([heads, GI], F32, name="rsb", tag="rsb")
        nc.vector.reciprocal(r_sb[:], d_sb[:])
        a_sb = apool.tile([heads, NG], F16, name="asb", tag="asb")
        nc.vector.tensor_mul(
            a_sb[:].rearrange("p (i n) -> p i n", i=GI),
            e_sb[:].rearrange("p (i n) -> p i n", i=GI),
            r_sb[:].rearrange("p (i o) -> p i o", o=1).broadcast_to((heads, GI, N)),
        )
        fillers = list(fillers)
        chunks = [range(0, 3), range(3, 6), range(6, 9), range(9, 12),
                  range(12, 15), range(15, 16)]
        for ci, chunk in enumerate(chunks):
            for jb in chunk:
                psum_a = pas.tile([128, NG], F32, name="psA", tag="pAS")
                nc.tensor.matmul(
                    psum_a[:],
                    mask2_sb[:, jb * 128 : (jb + 1) * 128],
                    a_sb[:],
                    start=True,
                    stop=True,
                )
                # p2 = a * v = psum_a * (vT8/48)
                p2 = apool.tile([128, NG], F16, name="p2", tag="p2")
                nc.vector.scalar_tensor_tensor(
                    p2[:], psum_a[:], 1.0 / SV, v3[:, jb],
                    op0=mybir.AluOpType.mult,
                    op1=mybir.AluOpType.mult,
                )
                ctx8 = apool.tile([128, GI], F32, name="ctx8", tag="ctx8")
                nc.vector.reduce_sum(
                    ctx8[:],
                    p2[:].rearrange("p (i n) -> p i n", i=GI),
                    axis=mybir.AxisListType.X,
                )
                nc.scalar.activation(
                    uT_v[:, jb, g * GI : (g + 1) * GI], ctx8[:],
                    mybir.ActivationFunctionType.Copy, scale=SU,
                )
            if ci < len(fillers):
                xpath_part(*fillers[ci])
        for fi in range(len(chunks), len(fillers)):
            xpath_part(*fillers[fi])

    def out0proj():
        # out0 = u @ Wc.T via fp8 DR: psum = (SU*ctx)@(SW*Wc) -> 1/(SU*SW)
        o0 = opool.tile([IPC, OUT], F16, name="o0sb", tag="o0sb")
        for oc in range(OC2):
            psum = po.tile([128, 512], F32, name="ps0", tag="pso")
            for k2 in range(KC2):
                nc.tensor.matmul(
                    psum[:IPC, :],
                    uT_v[:, 2 * k2 : 2 * k2 + 2, :],
                    wc8_v[:, 2 * k2 : 2 * k2 + 2, oc * 512 : (oc + 1) * 512],
                    start=(k2 == 0),
                    stop=(k2 == KC2 - 1),
                    perf_mode=mybir.MatmulPerfMode.DoubleRow,
                )
            nc.scalar.activation(
                o0[:, oc * 512 : (oc + 1) * 512], psum[:IPC, :],
                mybir.ActivationFunctionType.Copy, scale=1.0 / (SU * SW),
            )
        nc.sync.dma_start(out0_d.ap()[:], o0[:])

    # ---- schedule: fp8 vproj / attention pipelined with fp16 x-path tiles
    if variant == "full":
        load_x()
        vt0 = vproj(0, load_t8(0))
        s_prev = attnS(0, *vt0)
        v_prev = vt0[0]
        for g in range(1, G + 1):
            if g < G:
                vtg = vproj(g, load_t8(g))
            tiles = [3 * (g - 1) + t for t in range(3)]
            if g == G:
                tiles.append(12)
            fillers = [(t, p) for t in tiles for p in range(2)]
            attnAV(g - 1, v_prev, s_prev, fillers)
            if g < G:
                s_prev = attnS(g, *vtg)
                v_prev = vtg[0]
        out0proj()
    elif variant == "vproj":
        for g in range(G):
            vproj(g, load_t8(g))
    elif variant in ("xpath", "xmm", "xnodma"):
        load_x()
        if variant == "xmm":
            for ti in range(NT):
                for oc in range(OC2):
                    m0 = ti * 128
                    mw = min(128, XTOK - m0)
                    psum = po.tile([128, 512], F32, name="pso", tag="pso")
                    for kc in range(KC):
                        nc.tensor.matmul(
                            psum[:mw, :],
                            xstat(kc, m0, mw),
                            w2_v[:, kc, oc * 512 : (oc + 1) * 512],
                            start=(kc == 0),
                            stop=(kc == KC - 1),
                        )
        elif variant == "xnodma":
            for ti in range(NT):
                for oc in range(OC2):
                    m0 = ti * 128
                    mw = min(128, XTOK - m0)
                    psum = po.tile([128, 512], F32, name="pso", tag="pso")
                    for kc in range(KC):
                        nc.tensor.matmul(
                            psum[:mw, :],
                            xstat(kc, m0, mw),
                            w2_v[:, kc, oc * 512 : (oc + 1) * 512],
                            start=(kc == 0),
                            stop=(kc == KC - 1),
                        )
                    osb = opool.tile([128, OUT], F16, name="osb", tag="osb")
                    if oc == 0:
                        nc.vector.tensor_copy(osb[:mw, 0:512], psum[:mw, :])
                    else:
                        nc.scalar.activation(
                            osb[:mw, 512:1024], psum[:mw, :],
                            mybir.ActivationFunctionType.Copy,
                        )
        else:
            for ti in range(NT):
                xpath_tile(ti)
    elif variant == "attn":
        load_x()
        for g in range(G):
            vTg, pp8 = vproj(g, load_t8(g))
            attnAV(g, vTg, attnS(g, vTg, pp8))
        out0proj()


_NC_CACHE = {}
_RUN_CACHE = {}


def _get_nc(heads):
    if heads not in _NC_CACHE:
        _NC_CACHE[heads] = build_kernel(heads=heads)
    return _NC_CACHE[heads]


def _run(nc, in_maps):
    """run_bass_kernel_spmd equivalent (axon/PJRT path) with: the jitted
    executable cached across calls, weight-like inputs passed replicated
    (uploaded once, not 8x), and donated output buffers created on device
    (no zero upload)."""
    import jax
    import jax.numpy as jnp
    import numpy as _np
    from jax.sharding import Mesh, PartitionSpec, NamedSharding
    from jax.experimental.shard_map import shard_map
    import concourse.mybir as mb
    from concourse import bass2jax as b2j

    # inputs where every core got the identical array object -> replicated
    replicated = {
        nm
        for nm in in_maps[0]
        if all(m[nm] is in_maps[0][nm] for m in in_maps)
    }

    key = id(nc)
    if key not in _RUN_CACHE:
        b2j.install_neuronx_cc_hook()
        in_names, out_names, out_avals = [], [], []
        partition_name = (
            nc.partition_id_tensor.name if nc.partition_id_tensor else None
        )
        for alloc in nc.m.functions[0].allocations:
            if not isinstance(alloc, mb.MemoryLocationSet):
                continue
            name = alloc.memorylocations[0].name
            if alloc.kind == "ExternalInput":
                if name != partition_name:
                    in_names.append(name)
            elif alloc.kind == "ExternalOutput":
                shape = tuple(alloc.tensor_shape)
                dtype = mb.dt.np(alloc.dtype)
                out_names.append(name)
                out_avals.append(jax.core.ShapedArray(shape, dtype))
        n_params = len(in_names)
        n_outs = len(out_avals)
        all_names = list(in_names) + list(out_names)
        if partition_name is not None:
            all_names.append(partition_name)
        donate = tuple(range(n_params, n_params + n_outs))

        def _body(*args):
            operands = list(args)
            if partition_name is not None:
                operands.append(b2j.partition_id_tensor())
            outs = b2j._bass_exec_p.bind(
                *operands,
                out_avals=tuple(out_avals),
                in_names=tuple(all_names),
                out_names=tuple(out_names),
                lowering_input_output_aliases=(),
                sim_require_finite=True,
                sim_require_nnan=True,
                nc=nc,
            )
            return tuple(outs)

        devices = jax.devices()[:CORES]
        mesh = Mesh(_np.asarray(devices), ("core",))
        in_specs = tuple(
            PartitionSpec() if nm in replicated else PartitionSpec("core")
            for nm in in_names
        ) + (PartitionSpec("core"),) * n_outs
        out_specs = (PartitionSpec("core"),) * n_outs
        sharded = jax.jit(
            shard_map(
                _body, mesh=mesh, in_specs=in_specs, out_specs=out_specs,
                check_rep=False,
            ),
            donate_argnums=donate,
            keep_unused=True,
        )
        zeros_fns = [
            jax.jit(
                (lambda shape, dtype: lambda: jnp.zeros(shape, dtype))(
                    (CORES * av.shape[0], *av.shape[1:]), av.dtype
                ),
                out_shardings=NamedSharding(mesh, PartitionSpec("core")),
            )
            for av in out_avals
        ]
        _RUN_CACHE[key] = (
            sharded, in_names, out_names, out_avals, zeros_fns, replicated
        )

    sharded, in_names, out_names, out_avals, zeros_fns, replicated_c = (
        _RUN_CACHE[key]
    )
    assert replicated == replicated_c, "replication pattern changed"
    args = [
        _np.asarray(in_maps[0][nm])
        if nm in replicated
        else _np.concatenate([_np.asarray(m[nm]) for m in in_maps], axis=0)
        for nm in in_names
    ]
    dev_zeros = [f() for f in zeros_fns]
    out_arrs = sharded(*args, *dev_zeros)
    return [
        {
            nm: _np.asarray(out_arrs[i]).reshape(CORES, *out_avals[i].shape)[c]
            for i, nm in enumerate(out_names)
        }
        for c in range(CORES)
    ]


# ---------------------------------------------------------------- host side
def _fp8(a):
    f8np = mybir.dt.np(F8)  # ml_dtypes.float8_e4m3 (TRN range, max 240)
    return np.clip(a, -240.0, 240.0).astype(f8np)


def _chunkT(a):
    """[C, M] -> [128, KC*M]: row kc*128+p, col m -> [p, kc*M + m]."""
    Cdim, M = a.shape
    return np.ascontiguousarray(
        a.reshape(KC, 128, M).transpose(1, 0, 2).reshape(128, KC * M)
    )


_PREP_CACHE = {}


def make_in_maps(inputs, heads=H):
    x = np.asarray(inputs["x"], np.float32)
    pos_emb = np.asarray(inputs["pos_emb"], np.float32)
    Wv = np.asarray(inputs["Wv"], np.float32)
    bv = np.asarray(inputs["bv"], np.float32)
    Wc = np.asarray(inputs["Wc"], np.float32)
    bc = np.asarray(inputs["bc"], np.float32)
    num_heads = int(np.asarray(inputs["num_heads"]))
    assert num_heads == heads and x.shape == (B, C, S, S)
    assert 1 <= heads <= 128 and C % heads == 0

    wv8 = _fp8(_chunkT(64.0 * Wv.T))              # [128, KC*C]
    W2 = Wc @ Wv                                  # [OUT, C]
    w2T = _chunkT(W2.T).astype(np.float16)        # [128, KC*OUT]
    wc8 = _fp8(_chunkT(SW * Wc.T))                # [128, KC*OUT]

    # vposT[p, kc*N + n] = SV * (pos_emb @ Wv.T + bv)[n, kc*128+p]
    vpos = SV * (pos_emb @ Wv.T + bv)             # [N, C]
    vposT = _chunkT(vpos.T).astype(np.float16)

    # maskT[p, kc*heads + h] = 1 if channel kc*128+p belongs to head h
    head_of = np.arange(C) // (C // heads)
    maskT = np.zeros((128, KC * heads), mybir.dt.np(F8))
    mask2 = np.zeros((heads, KC * 128), np.float16)
    for kc in range(KC):
        for p in range(128):
            h = head_of[kc * 128 + p]
            maskT[p, kc * heads + h] = 1.0
            mask2[h, kc * 128 + p] = 1.0

    # x tokens, channel-chunk-major: xT[p, kc, j] = x[j//49, kc*128+p, j%49]
    xt = x.reshape(B, KC, 128, HW)
    # t8[p, (g, kc, i, n)] = fp8(32*t), t = [mean(x); x_tokens] (pos excl.)
    tok = x.reshape(B, C, HW).transpose(0, 2, 1)            # [B, HW, C]
    tt = np.concatenate([tok.mean(axis=1, keepdims=True), tok], axis=1)
    q8 = _fp8(32.0 * tt)                                    # [B, N, C]

    in_maps = []
    for core in range(CORES):
        sl = slice(core * IPC, (core + 1) * IPC)
        xT = np.ascontiguousarray(
            xt[sl].transpose(2, 1, 0, 3).reshape(128, KC, XTOK)
        ).astype(np.float16)
        t8c = q8[sl].reshape(G, GI, N, KC, 128)
        t8c = np.ascontiguousarray(
            t8c.transpose(4, 0, 3, 1, 2).reshape(128, G * KC * NG)
        )
        in_maps.append(
            {
                "xa": np.ascontiguousarray(
                    xT[:, :, :XA].reshape(128, KC * XA)),
                "xb": np.ascontiguousarray(
                    xT[:, :, XA:].reshape(128, KC * XB)),
                "t8": t8c,
                "wv8": wv8,
                "w2T": w2T,
                "wc8": wc8,
                "vposT": vposT,
                "maskT": maskT,
                "mask2": mask2,
            }
        )

    return in_maps


def kernel(**inputs):
    from concourse._compat import axon_active

    heads = int(np.asarray(inputs["num_heads"]))
    in_maps = make_in_maps(inputs, heads)
    nc = _get_nc(heads)
    if axon_active():
        results = _run(nc, in_maps)
    else:
        results = run_bass_kernel_spmd(nc, in_maps, list(range(CORES))).results

    pos_emb = np.asarray(inputs["pos_emb"], np.float32)
    Wv = np.asarray(inputs["Wv"], np.float32)
    bv = np.asarray(inputs["bv"], np.float32)
    Wc = np.asarray(inputs["Wc"], np.float32)
    bc = np.asarray(inputs["bc"], np.float32)
    pconst = pos_emb @ (Wc @ Wv).T + bv @ Wc.T + bc      # [N, OUT]

    out = np.empty((B, N, OUT), np.float32)
    for i in range(CORES):
        blk = out[i * IPC : (i + 1) * IPC]
        blk[:, 1:] = np.asarray(results[i]["outx"], np.float32).reshape(
            IPC, HW, OUT)
        blk[:, 0] = np.asarray(results[i]["out0"], np.float32)
    out[:, 1:, :] += pconst[None, 1:, :]
    out[:, 0, :] += bc[None, :]
    return out
